# revision 1
# baseline (speedup 1.0000x reference)
"""Distributed Trainium2 kernel for a transformer attention block (B=2, S=4096,
D=1024, H=4096, fp32 I/O).

Reference computation (note the Q<-k, K<-q, V<-v argument quirk):
    k = x @ Wk + bk ; q = x @ Wq + bq ; v = x @ Wv + bv
    scores = (k @ q^T) / sqrt(D); attn = softmax(scores) @ v
    x1 = LN(x + attn); h = gelu(x1 @ W1 + b1); out = LN(x1 + h @ W2 + b2)

Sharding: 8 cores -> 2 groups of 4 (one group per batch element); each core
owns 1024 sequence rows. Activations live transposed ([d, s] with d on SBUF
partitions) so every GEMM consumes the weights exactly as stored. Each core
computes its local q^T / v; two bf16 AllGathers within the 4-core group
provide the full-sequence q (keys) and v (values). Softmax runs without max
subtraction (|scores| < ~2 here); row sums come from ones-vector matmuls on
the TensorEngine and normalization is deferred until after the attention GEMM.
LayerNorm stats (partition-axis reductions in this layout) also use
ones-matmuls, with rank-1 PE matmuls broadcasting per-column stats back across
partitions. Compute dtype is bf16 with fp32 PSUM accumulation; the residual
stream stays fp32 in a single set of in-place tiles.
"""

import sys

if "/opt/trn_rl_repo" not in sys.path:
    sys.path.insert(0, "/opt/trn_rl_repo")

import numpy as np

import concourse.bacc as bacc
import concourse.mybir as mybir
import concourse.tile as tile
from concourse.alu_op_type import AluOpType
from concourse.masks import make_identity


AF = mybir.ActivationFunctionType
FP32 = mybir.dt.float32
BF16 = mybir.dt.bfloat16
FP8 = mybir.dt.float8e4

B, S, D, H = 2, 4096, 1024, 4096
N_CORES = 8
G = 4                 # cores per group (one group per batch element)
S_LOC = S // G        # sequence rows per core
P = 128               # SBUF partitions
NF = 512              # matmul moving free-dim (one fp32 PSUM bank)
DT = D // P           # 8 d-tiles
ST = S_LOC // P       # 8 s-tiles per core
HT = H // P           # 32 h-tiles
HG = HT // 4          # 8 h-tiles per fused FFN group
EPS = 1e-5
SM_SCALE = 1.0 / float(np.sqrt(np.float32(D)))

GROUPS = [[0, 1, 2, 3], [4, 5, 6, 7]]


def build_graph(nc, tc, ext):
    mm_pool = ext["mm_pool"]
    stream = ext["stream"]
    persist = ext["persist"]
    stage = ext["stage"]
    const = ext["const"]
    dram = ext["dram"]
    tcx = ext["tc"]

    # ---- constants ----
    ident = const.tile([P, P], FP32, tag="ident", name="ident")
    make_identity(nc, ident[:])
    ones_bf = const.tile([P, P], BF16, tag="ones_bf", name="ones_bf")
    nc.vector.memset(ones_bf[:], 1.0)
    ones_f32 = const.tile([P, P], FP32, tag="ones_f32", name="ones_f32")
    nc.vector.memset(ones_f32[:], 1.0)
    eps_t = const.tile([1, 1], FP32, tag="eps", name="eps")
    nc.vector.memset(eps_t[:], EPS)

    # all per-partition bias/scale vectors packed into one tile (a [128, 1]
    # tile pads to ~16KB; sixty of them would waste ~1MB)
    pvecs = const.tile([P, 80], FP32, tag="pvecs", name="pvecs")
    _pvec_col = [0]

    def load_pvec(ext_t, n_tiles, name):
        tiles = []
        for m in range(n_tiles):
            c = _pvec_col[0]
            _pvec_col[0] += 1
            sl = pvecs[:, c:c + 1]
            nc.sync.dma_start(out=sl, in_=ext_t[m * P:(m + 1) * P, 0:1])
            tiles.append(sl)
        return tiles

    bq_sb = load_pvec(ext["bq_ext"], DT, "bq")
    bk_sb = load_pvec(ext["bk_ext"], DT, "bk")
    b1_sb = load_pvec(ext["b1_ext"], HT, "b1")
    b2_sb = load_pvec(ext["b2_ext"], DT, "b2")
    gamma_sb = load_pvec(ext["gamma_ext"], DT, "gamma")
    beta_sb = load_pvec(ext["beta_ext"], DT, "beta")

    # "smalls": one tile hosting the [1, N] vectors (each would otherwise burn
    # a full free-size strip across all 128 partitions). Rows are 32-aligned so
    # they can feed matmul operands.
    smalls = const.tile([P, D], FP32, tag="smalls", name="smalls")
    bv_row = smalls[0:1, :]
    nc.sync.dma_start(out=bv_row, in_=ext["bv_ext"][0:1, :])
    bv_b = const.tile([P, D], FP32, tag="bv_b", name="bv_b")
    for n0 in range(0, D, NF):
        pt = mm_pool.tile([P, NF], FP32, tag="mm", name="mm")
        nc.tensor.matmul(pt[:], ones_f32[0:1, :], bv_row[:, n0:n0 + NF])
        nc.scalar.copy(out=bv_b[:, n0:n0 + NF], in_=pt[:])

    # ---- load x, transpose to x^T (bf16; also the residual source) ----
    xT_bf = [persist.tile([P, S_LOC], BF16, tag=f"bfA{d}", name=f"bfA{d}") for d in range(DT)]
    for si in range(ST):
        xn = stage.tile([P, D], FP32, tag="stgf", name="stgf")
        nc.sync.dma_start(out=xn[:], in_=ext["x_ext"][si * P:(si + 1) * P, :])
        for dj in range(DT):
            pt = mm_pool.tile([P, P], FP32, tag="mm", name="mm")
            nc.tensor.transpose(pt[:], xn[:, dj * P:(dj + 1) * P], ident[:])
            nc.vector.tensor_copy(out=xT_bf[dj][:, si * P:(si + 1) * P], in_=pt[:])

    # ---- helper: stream a [D, D] weight into bf16 tiles (shared tag family) ----
    def load_weight_bf(ext_t, row0=0):
        tiles = []
        for kd in range(DT):
            wf = stage.tile([P, D], FP32, tag="stgf", name="stgf")
            nc.sync.dma_start(
                out=wf[:], in_=ext_t[row0 + kd * P:row0 + (kd + 1) * P, :]
            )
            wb = stream.tile([P, D], BF16, tag=f"str{kd}", name=f"str{kd}")
            nc.vector.tensor_copy(out=wb[:], in_=wf[:])
            tiles.append(wb)
        return tiles

    # ---- QKV projections ----
    def proj_T(w_tiles, bias_tiles, fam, dtype=BF16):
        outs = []
        for m in range(DT):
            pt = mm_pool.tile([P, S_LOC], FP32, tag="mm", name="mm")
            for n0 in range(0, S_LOC, NF):
                for kd in range(DT):
                    nc.tensor.matmul(
                        pt[:, n0:n0 + NF],
                        w_tiles[kd][:, m * P:(m + 1) * P],
                        xT_bf[kd][:, n0:n0 + NF],
                        start=(kd == 0), stop=(kd == DT - 1),
                    )
            o = persist.tile([P, S_LOC], dtype, tag=f"{fam}{m}", name=f"{fam}{m}")
            nc.scalar.activation(o[:], pt[:], AF.Identity, bias=bias_tiles[m])
            outs.append(o)
        return outs

    # q^T[dout, s] = Wq.T @ x^T   (lhsT = Wq as stored)
    wq_bf = load_weight_bf(ext["wq_ext"])
    qT_bf = proj_T(wq_bf, bq_sb, "bfB", dtype=FP8)

    ag_q_in = dram.tile([D, S_LOC], FP8, name="agqi")
    for m in range(DT):
        nc.sync.dma_start(out=ag_q_in[m * P:(m + 1) * P, :], in_=qT_bf[m][:])
    ag_q_out = dram.tile([G * D, S_LOC], FP8, name="agqo")
    nc.gpsimd.collective_compute(
        "AllGather", AluOpType.bypass, replica_groups=GROUPS,
        ins=[ag_q_in[:].opt()], outs=[ag_q_out[:].opt()],
    )

    # v[t, d] = x^T.T @ Wv   (lhsT = x^T tiles, rhs = Wv)
    wv_bf = load_weight_bf(ext["wv_ext"])
    v_bf = []
    for mt in range(ST):
        pt = mm_pool.tile([P, D], FP32, tag="mm", name="mm")
        for n0 in range(0, D, NF):
            for kd in range(DT):
                nc.tensor.matmul(
                    pt[:, n0:n0 + NF],
                    xT_bf[kd][:, mt * P:(mt + 1) * P],
                    wv_bf[kd][:, n0:n0 + NF],
                    start=(kd == 0), stop=(kd == DT - 1),
                )
        o = persist.tile([P, D], BF16, tag=f"bfC{mt}", name=f"bfC{mt}")
        nc.vector.tensor_add(o[:], pt[:], bv_b[:])
        v_bf.append(o)

    ag_v_in = dram.tile([S_LOC, D], BF16, name="agvi")
    for mt in range(ST):
        nc.sync.dma_start(out=ag_v_in[mt * P:(mt + 1) * P, :], in_=v_bf[mt][:])
    ag_v_out = dram.tile([G * S_LOC, D], BF16, name="agvo")
    nc.gpsimd.collective_compute(
        "AllGather", AluOpType.bypass, replica_groups=GROUPS,
        ins=[ag_v_in[:].opt()], outs=[ag_v_out[:].opt()],
    )

    # k^T (overlaps the collectives)
    wk_bf = load_weight_bf(ext["wk_ext"])
    kT_bf = proj_T(wk_bf, bk_sb, "bfD")

    # ---- attention; res[m] <- x^T + attn^T (normalized), fp32, in place ----
    # Both score passes run before the attention passes: scores need only the
    # q AllGather, so pass A of BOTH s-halves covers the v AllGather's wire
    # time. Half 1's P tiles are packed pairwise into the dead q^T / local-v
    # bf16 slots (bfB/bfC) so this costs no extra SBUF.
    res = [persist.tile([P, S_LOC], FP32, tag=f"res{m}", name=f"res{m}") for m in range(DT)]

    p_tiles = [{}, {}]
    packed_h1 = {}
    # rs accumulators and reciprocals live in the smalls tile:
    #   row 32: rowsum[half] at columns half*NF; rows 64/96: recip[half]
    rs_sl = [smalls[32:33, h * NF:(h + 1) * NF] for h in range(2)]
    recip_sl = [smalls[64:65, h * NF:(h + 1) * NF] for h in range(2)]

    # pass A, both halves per chunk (each q chunk is loaded exactly once)
    for r in range(G):
        qch = []
        for dsub in range(DT):
            q8 = stream.tile([P, S_LOC], FP8, tag=f"str{dsub}", name=f"str{dsub}")
            nc.sync.dma_start(
                out=q8[:],
                in_=ag_q_out[r * D + dsub * P:r * D + (dsub + 1) * P, :],
            )
            qt = stream.tile([P, S_LOC], BF16, tag=f"str{dsub}", name=f"str{dsub}")
            nc.vector.tensor_copy(out=qt[:], in_=q8[:])
            qch.append(qt)
        for half in range(2):
            n0 = half * NF
            for tt in range(ST):
                ps = mm_pool.tile([P, NF], FP32, tag="mm", name="mm")
                for dsub in range(DT):
                    nc.tensor.matmul(
                        ps[:],
                        qch[dsub][:, tt * P:(tt + 1) * P],
                        kT_bf[dsub][:, n0:n0 + NF],
                        start=(dsub == 0), stop=(dsub == DT - 1),
                    )
                if half == 0:
                    pt = persist.tile([P, NF], BF16, tag=f"P{r}_{tt}",
                                      name=f"P{r}_{tt}")
                else:
                    idx = r * (ST // 2) + tt // 2
                    if idx not in packed_h1:
                        fam = "bfB" if idx < 8 else "bfC"
                        packed_h1[idx] = persist.tile(
                            [P, S_LOC], BF16, tag=f"{fam}{idx % 8}",
                            name=f"{fam}{idx % 8}",
                        )
                    pt = packed_h1[idx][:, (tt % 2) * NF:(tt % 2 + 1) * NF]
                nc.scalar.activation(pt[:], ps[:], AF.Exp, scale=SM_SCALE)
                p_tiles[half][(r, tt)] = pt
            # per-chunk row sums (one-shot so the PSUM slot is short-lived)
            rs_ps = mm_pool.tile([1, NF], FP32, tag="mm", name="mm")
            for tt in range(ST):
                nc.tensor.matmul(
                    rs_ps[:], ones_bf[:, 0:1], p_tiles[half][(r, tt)][:],
                    start=(tt == 0), stop=(tt == ST - 1),
                )
            if r == 0:
                nc.vector.tensor_copy(out=rs_sl[half], in_=rs_ps[:])
            else:
                nc.vector.tensor_add(rs_sl[half], rs_ps[:], rs_sl[half])

    recip_bs = []
    for half in range(2):
        nc.vector.reciprocal(recip_sl[half], rs_sl[half])
        rb_ps = mm_pool.tile([P, NF], FP32, tag="mm", name="mm")
        nc.tensor.matmul(rb_ps[:], ones_f32[64:65, :], recip_sl[half])
        recip_b = stage.tile([P, NF], FP32, tag=f"bc{half}",
                             name=f"bc{half}", bufs=1)
        nc.scalar.copy(out=recip_b[:], in_=rb_ps[:])
        recip_bs.append(recip_b)

    for half in range(2):
        n0 = half * NF
        # pass B: attn^T[d, s] = v.T @ P^T, 2 m-groups of 4 PSUM accumulators
        with tcx.tile_pool(name=f"at{half}", bufs=1, space="PSUM") as at_pool:
            for mg in range(2):
                at_ps = [
                    at_pool.tile([P, NF], FP32, tag=f"at{i}", name=f"at{i}") for i in range(4)
                ]
                for r in range(G):
                    vch = []
                    for tt in range(ST):
                        vt = stream.tile([P, D], BF16, tag=f"str{tt}", name=f"str{tt}")
                        nc.sync.dma_start(
                            out=vt[:],
                            in_=ag_v_out[
                                r * S_LOC + tt * P:r * S_LOC + (tt + 1) * P, :
                            ],
                        )
                        vch.append(vt)
                    for mi in range(4):
                        m = mg * 4 + mi
                        for tt in range(ST):
                            nc.tensor.matmul(
                                at_ps[mi][:],
                                vch[tt][:, m * P:(m + 1) * P],
                                p_tiles[half][(r, tt)][:],
                                start=(r == 0 and tt == 0),
                                stop=(r == G - 1 and tt == ST - 1),
                            )
                for mi in range(4):
                    m = mg * 4 + mi
                    u = stage.tile([P, NF], FP32, tag="tmp", name="tmp", bufs=2)
                    nc.vector.tensor_mul(u[:], at_ps[mi][:], recip_bs[half][:])
                    nc.vector.tensor_add(
                        res[m][:, n0:n0 + NF], u[:], xT_bf[m][:, n0:n0 + NF]
                    )

    # ---- layernorm over d (partition axis) applied in place to res ----
    ln_counter = [0]

    def layer_norm_T(out_bf=None):
        ln_counter[0] += 1
        with tcx.tile_pool(
            name=f"ln{ln_counter[0]}", bufs=1, space="PSUM"
        ) as ln_pool:
            for n0 in range(0, S_LOC, NF):
                sum_ps = ln_pool.tile([1, NF], FP32, tag="ln_sum", name="ln_sum")
                for m in range(DT):
                    nc.tensor.matmul(
                        sum_ps[:], ones_f32[:, 0:1], res[m][:, n0:n0 + NF],
                        start=(m == 0), stop=(m == DT - 1),
                    )
                sq_ps = ln_pool.tile([1, NF], FP32, tag="ln_sq", name="ln_sq")
                for m in range(DT):
                    sq = stage.tile([P, NF], FP32, tag="tmp", name="tmp", bufs=2)
                    nc.vector.tensor_mul(
                        sq[:], res[m][:, n0:n0 + NF], res[m][:, n0:n0 + NF]
                    )
                    nc.tensor.matmul(
                        sq_ps[:], ones_f32[:, 0:1], sq[:],
                        start=(m == 0), stop=(m == DT - 1),
                    )
                negmu = stage.tile([1, NF], FP32, tag="ln_negmu", name="ln_negmu", bufs=1)
                nc.vector.tensor_scalar_mul(negmu[:], sum_ps[:], -1.0 / D)
                m2 = stage.tile([1, NF], FP32, tag="ln_m2", name="ln_m2", bufs=1)
                nc.vector.tensor_scalar_mul(m2[:], sq_ps[:], 1.0 / D)
                musq = stage.tile([1, NF], FP32, tag="ln_musq", name="ln_musq", bufs=1)
                nc.vector.tensor_mul(musq[:], negmu[:], negmu[:])
                nc.vector.tensor_sub(m2[:], m2[:], musq[:])      # m2 <- var
                nc.scalar.activation(musq[:], m2[:], AF.Sqrt, bias=eps_t[:])
                rstd = musq                                       # musq <- sd -> rstd
                nc.vector.reciprocal(rstd[:], rstd[:])

                negmu_b = stage.tile([P, NF], FP32, tag="bc0", name="bc0", bufs=1)
                rstd_b = stage.tile([P, NF], FP32, tag="bc1", name="bc1", bufs=1)
                for src, dst in ((negmu, negmu_b), (rstd, rstd_b)):
                    bp = mm_pool.tile([P, NF], FP32, tag="mm", name="mm")
                    nc.tensor.matmul(bp[:], ones_f32[0:1, :], src[0:1, :])
                    nc.scalar.copy(out=dst[:], in_=bp[:])

                for m in range(DT):
                    t = stage.tile([P, NF], FP32, tag="tmp", name="tmp", bufs=2)
                    nc.vector.tensor_add(t[:], res[m][:, n0:n0 + NF], negmu_b[:])
                    t2 = stage.tile([P, NF], FP32, tag="tmp", name="tmp", bufs=2)
                    nc.vector.tensor_mul(t2[:], t[:], rstd_b[:])
                    nc.vector.tensor_scalar(
                        res[m][:, n0:n0 + NF], t2[:],
                        gamma_sb[m], beta_sb[m],
                        op0=AluOpType.mult, op1=AluOpType.add,
                    )
                    if out_bf is not None:
                        nc.scalar.copy(
                            out=out_bf[n0 // NF][m][:],
                            in_=res[m][:, n0:n0 + NF],
                        )

    # bf16 copy of x1 for the FFN GEMMs (reuses the q^T family); slices are
    # written inside the LN apply loop so FFN1 can start immediately after.
    x1_bh = [
        [persist.tile([P, NF], BF16, tag=f"bfB{m}", name=f"bfB{m}")
         for m in range(DT)],
        [persist.tile([P, NF], BF16, tag=f"bfD{m}", name=f"bfD{m}")
         for m in range(DT)],
    ]

    # prefetch the first FFN weight group during LN1
    w1g0 = []
    for kd in range(DT):
        wf = stage.tile([P, HG * P], FP32, tag="stgf", name="stgf")
        nc.sync.dma_start(
            out=wf[:], in_=ext["w1_ext"][kd * P:(kd + 1) * P, 0:HG * P]
        )
        wb = stream.tile([P, HG * P], BF16, tag=f"str{kd}", name=f"str{kd}")
        nc.vector.tensor_copy(out=wb[:], in_=wf[:])
        w1g0.append(wb)

    layer_norm_T(out_bf=x1_bh)  # res <- x1 (fp32)

    # ---- fused FFN: per h-group, FFN1 -> gelu -> FFN2 partial into res ----
    for g in range(HT // HG):
        if g == 0:
            w1g = w1g0
        else:
            w1g = []
            for kd in range(DT):
                wf = stage.tile([P, HG * P], FP32, tag="stgf", name="stgf")
                nc.sync.dma_start(
                    out=wf[:],
                    in_=ext["w1_ext"][kd * P:(kd + 1) * P,
                                      g * HG * P:(g + 1) * HG * P],
                )
                wb = stream.tile([P, HG * P], BF16, tag=f"str{kd}", name=f"str{kd}")
                nc.vector.tensor_copy(out=wb[:], in_=wf[:])
                w1g.append(wb)
        hT = []
        famh = "bfA" if g % 2 == 0 else "bfC"
        for mh_i in range(HG):
            mh = g * HG + mh_i
            pt = mm_pool.tile([P, S_LOC], FP32, tag="mm", name="mm")
            for n0 in range(0, S_LOC, NF):
                for kd in range(DT):
                    nc.tensor.matmul(
                        pt[:, n0:n0 + NF],
                        w1g[kd][:, mh_i * P:(mh_i + 1) * P],
                        x1_bh[n0 // NF][kd][:],
                        start=(kd == 0), stop=(kd == DT - 1),
                    )
            ht = persist.tile([P, S_LOC], BF16, tag=f"{famh}{mh_i}", name=f"{famh}{mh_i}")
            nc.scalar.activation(ht[:], pt[:], AF.Gelu, bias=b1_sb[mh])
            hT.append(ht)

        w2g = []
        for kh_i in range(HG):
            wf = stage.tile([P, D], FP32, tag="stgf", name="stgf")
            nc.sync.dma_start(
                out=wf[:],
                in_=ext["w2_ext"][g * HG * P + kh_i * P:
                                  g * HG * P + (kh_i + 1) * P, :],
            )
            wb = stream.tile([P, D], BF16, tag=f"str{kh_i}", name=f"str{kh_i}")
            nc.vector.tensor_copy(out=wb[:], in_=wf[:])
            w2g.append(wb)
        for m in range(DT):
            pt = mm_pool.tile([P, S_LOC], FP32, tag="mm", name="mm")
            for n0 in range(0, S_LOC, NF):
                for kh_i in range(HG):
                    nc.tensor.matmul(
                        pt[:, n0:n0 + NF],
                        w2g[kh_i][:, m * P:(m + 1) * P],
                        hT[kh_i][:, n0:n0 + NF],
                        start=(kh_i == 0), stop=(kh_i == HG - 1),
                    )
            for n0 in range(0, S_LOC, NF):
                if g == 0:
                    # res <- (ffn2 + b2) + x1
                    nc.vector.scalar_tensor_tensor(
                        out=res[m][:, n0:n0 + NF], in0=pt[:, n0:n0 + NF],
                        scalar=b2_sb[m], in1=res[m][:, n0:n0 + NF],
                        op0=AluOpType.add, op1=AluOpType.add,
                    )
                else:
                    nc.vector.tensor_add(
                        res[m][:, n0:n0 + NF], pt[:, n0:n0 + NF],
                        res[m][:, n0:n0 + NF],
                    )

    layer_norm_T()  # res <- out^T (fp32)

    # ---- transpose back to [s, d] and store ----
    for si in range(ST):
        onat = stage.tile([P, D], FP32, tag="stgf", name="stgf")
        for dj in range(DT):
            pt = mm_pool.tile([P, P], FP32, tag="mm", name="mm")
            nc.tensor.transpose(pt[:], res[dj][:, si * P:(si + 1) * P], ident[:])
            nc.scalar.copy(out=onat[:, dj * P:(dj + 1) * P], in_=pt[:])
        nc.sync.dma_start(out=ext["out_ext"][si * P:(si + 1) * P, :], in_=onat[:])


def build_nc():
    nc = bacc.Bacc(target_bir_lowering=False, num_devices=N_CORES)

    ext = {
        "x_ext": nc.declare_dram_parameter("x", [S_LOC, D], FP32, isOutput=False),
        "wq_ext": nc.declare_dram_parameter("Wq", [D, D], FP32, isOutput=False),
        "wk_ext": nc.declare_dram_parameter("Wk", [D, D], FP32, isOutput=False),
        "wv_ext": nc.declare_dram_parameter("Wv", [D, D], FP32, isOutput=False),
        "w1_ext": nc.declare_dram_parameter("W1", [D, H], FP32, isOutput=False),
        "w2_ext": nc.declare_dram_parameter("W2", [H, D], FP32, isOutput=False),
        "bq_ext": nc.declare_dram_parameter("bq", [D, 1], FP32, isOutput=False),
        "bk_ext": nc.declare_dram_parameter("bk", [D, 1], FP32, isOutput=False),
        "bv_ext": nc.declare_dram_parameter("bv", [1, D], FP32, isOutput=False),
        "b1_ext": nc.declare_dram_parameter("b1", [H, 1], FP32, isOutput=False),
        "b2_ext": nc.declare_dram_parameter("b2", [D, 1], FP32, isOutput=False),
        "gamma_ext": nc.declare_dram_parameter("gamma", [D, 1], FP32, isOutput=False),
        "beta_ext": nc.declare_dram_parameter("beta", [D, 1], FP32, isOutput=False),
        "out_ext": nc.declare_dram_parameter("out", [S_LOC, D], FP32, isOutput=True),
    }

    with tile.TileContext(nc) as tc:
        with (
            tc.tile_pool(name="dram", bufs=1, space="DRAM") as dram,
            tc.tile_pool(name="const", bufs=1) as const,
            tc.tile_pool(name="persist", bufs=1) as persist,
            tc.tile_pool(name="stage", bufs=2) as stage,
            tc.tile_pool(name="stream", bufs=3) as stream,
            tc.tile_pool(name="mm", bufs=2, space="PSUM") as mm_pool,
        ):
            ext.update(
                tc=tc, dram=dram, const=const, persist=persist,
                stage=stage, stream=stream, mm_pool=mm_pool,
            )
            build_graph(nc, tc, ext)
    nc.compile()
    return nc


_NC_CACHE = None


def _get_nc():
    global _NC_CACHE
    if _NC_CACHE is None:
        _NC_CACHE = build_nc()
    return _NC_CACHE


def _make_in_maps(inputs):
    x = np.asarray(inputs["input_embedding"], dtype=np.float32)
    assert x.shape == (B, S, D), x.shape

    shared = {
        "Wq": np.ascontiguousarray(inputs["Wq"], np.float32),
        "Wk": np.ascontiguousarray(inputs["Wk"], np.float32),
        "Wv": np.ascontiguousarray(inputs["Wv"], np.float32),
        "W1": np.ascontiguousarray(inputs["W1"], np.float32),
        "W2": np.ascontiguousarray(inputs["W2"], np.float32),
        "bq": np.asarray(inputs["bq"], np.float32).reshape(D, 1),
        "bk": np.asarray(inputs["bk"], np.float32).reshape(D, 1),
        "bv": np.asarray(inputs["bv"], np.float32).reshape(1, D),
        "b1": np.asarray(inputs["b1"], np.float32).reshape(H, 1),
        "b2": np.asarray(inputs["b2"], np.float32).reshape(D, 1),
        "gamma": np.asarray(inputs["gamma"], np.float32).reshape(D, 1),
        "beta": np.asarray(inputs["beta"], np.float32).reshape(D, 1),
    }

    in_maps = []
    for c in range(N_CORES):
        b = c // G
        r = c % G
        m = dict(shared)
        m["x"] = np.ascontiguousarray(x[b, r * S_LOC:(r + 1) * S_LOC, :])
        in_maps.append(m)
    return in_maps


def kernel(**inputs: np.ndarray) -> np.ndarray:
    from concourse.bass_utils import run_bass_kernel_spmd

    in_maps = _make_in_maps(inputs)
    nc = _get_nc()
    res = run_bass_kernel_spmd(nc, in_maps, core_ids=list(range(N_CORES)))

    out = np.empty((B, S, D), dtype=np.float32)
    for c in range(N_CORES):
        b = c // G
        r = c % G
        out[b, r * S_LOC:(r + 1) * S_LOC, :] = res.results[c]["out"]
    return out

